# revision 12
# baseline (speedup 1.0000x reference)
"""Encoder-decoder attention kernel for Trainium2, 8 NeuronCores.

Sharding: batch (B=8) data-parallel, one batch element per core; weights
replicated. Per core (S=Sq=Sk=1024, H=1024, NH=16, D=64):

All matmuls run in fp16 (1 cycle/row on PE) with fp32 PSUM accumulation.
Host ships X_dec^T, X_enc^T and all weights pre-transposed + pre-cast to
fp16 (1/sqrt(D)=1/8 folded into W_query), so there is no on-chip
transpose/cast phase for the inputs.

Per head-pair p (heads 2p, 2p+1 stacked as 64+64 partitions):
  Q^T = Wq_p^T.T @ X_dec^T  -> PSUM -> fp16 qt [128, S]
  K^T -> fp16 kt [128, S];  V -> v2 [128 k, kt, 512 nd] fp16 (per 4-pair grp)
  scores: contraction-64 fp16 matmuls, ev head on PE rows 0-63
    (tile_position (0,0)) and od head on rows 64-127 ((64,0)) so the two
    heads' streams overlap in the array.
  softmax per [128 q, 1024 k] tile:
    DVE tensor_tensor_reduce: out=-max(s_lo,s_hi) elemwise (fp16 scratch),
      min-accum -> negmax (one 512-elem pass instead of a 1024-elem reduce)
    ACT exp(bias=negmax, accum_out=rowsum) -> p_e fp16
    DVE reciprocal + tensor_scalar_mul (4x fp16) -> p_h
    sync DMA transpose p_h -> pt [128 k, kt, q]
  PV: fp16, ev/od col-split via tile_position (0,0)/(0,64)
out = concat @ W_out^T + b_out (fp16 matmuls, DVE bias add).

Error budget: fp16 rounding of q/k gives score error ~0.1 abs (scores are
~N(0, 2000^2), top-2 gaps ~400); measured end-to-end rel l2 ~1.5e-3 vs
the 2e-2 gate.
"""
import sys

sys.path.insert(0, "/opt/trn_rl_repo")

import numpy as np

B = 8
S = 1024   # Sq == Sk
H = 1024
NH = 16
D = 64
P = 128
HT = H // P    # 8 h-tiles
ST = S // P    # 8 s-tiles == k-tiles
NP = NH // 2   # 8 head pairs
QB = 256       # q-block width for the P@V moving dim
NB = S // QB   # 4 q-blocks
QTB = QB // P  # 2 q-tiles per block


def build():
    import concourse.mybir as mybir
    import concourse.tile as tile
    from concourse import bacc

    f32 = mybir.dt.float32
    f16 = mybir.dt.float16
    AX = mybir.AxisListType.X
    OP = mybir.AluOpType
    AF = mybir.ActivationFunctionType

    nc = bacc.Bacc(trn_type="TRN2", target_bir_lowering=False, debug=False)

    xdt_d = nc.dram_tensor("xdt", [H, S], f16, kind="ExternalInput").ap()   # X_dec^T
    xet_d = nc.dram_tensor("xet", [H, S], f16, kind="ExternalInput").ap()   # X_enc^T
    wqt_d = nc.dram_tensor("wqt", [H, H], f16, kind="ExternalInput").ap()   # [h, nd] (pre-scaled 1/8)
    wkt_d = nc.dram_tensor("wkt", [H, H], f16, kind="ExternalInput").ap()   # [h, nd]
    wvt_d = nc.dram_tensor("wvt", [H, H], f16, kind="ExternalInput").ap()   # [h, nd]
    wot_d = nc.dram_tensor("wot", [H, H], f16, kind="ExternalInput").ap()   # [nd, h_out]
    bias_d = nc.dram_tensor("bias", [P, H], f32, kind="ExternalInput").ap()
    out_d = nc.dram_tensor("out", [S, H], f32, kind="ExternalOutput").ap()

    from contextlib import ExitStack
    with tile.TileContext(nc) as tc:
        with ExitStack() as ctx:
            xp = ctx.enter_context(tc.tile_pool(name="x", bufs=16))
            wp = ctx.enter_context(tc.tile_pool(name="w", bufs=24))
            qtp = ctx.enter_context(tc.tile_pool(name="qt", bufs=3))
            ktp = ctx.enter_context(tc.tile_pool(name="kt", bufs=3))
            vpp = ctx.enter_context(tc.tile_pool(name="vp", bufs=2))
            ccp = ctx.enter_context(tc.tile_pool(name="cc", bufs=NP))
            pep = ctx.enter_context(tc.tile_pool(name="pe", bufs=6))
            php = ctx.enter_context(tc.tile_pool(name="ph", bufs=6))
            ptp = ctx.enter_context(tc.tile_pool(name="pt", bufs=6))
            wop = ctx.enter_context(tc.tile_pool(name="wo", bufs=4))
            osbp = ctx.enter_context(tc.tile_pool(name="osb", bufs=2))
            constp = ctx.enter_context(tc.tile_pool(name="const", bufs=1))
            statp = ctx.enter_context(tc.tile_pool(name="stat", bufs=24))
            psp = ctx.enter_context(tc.tile_pool(name="ps", bufs=2, space="PSUM"))
            psSp = ctx.enter_context(tc.tile_pool(name="psS", bufs=3, space="PSUM"))

            def pstile():
                return psp.tile([P, 512], f32, tag="ps", name="ps")

            def pstileS():
                return psSp.tile([P, S], f32, tag="psS", name="psS")

            def stat():
                return statp.tile([P, 1], f32, tag="stat", name="stat")

            # ---- constants / preloaded inputs ----
            bias_sb = constp.tile([P, H], f32)
            nc.gpsimd.dma_start(bias_sb[:], bias_d)

            xdt_sb = [xp.tile([P, S], f16, tag="x", name="xdt") for _ in range(HT)]
            xet_sb = [xp.tile([P, S], f16, tag="x", name="xet") for _ in range(HT)]
            wq_sb = [wp.tile([P, H], f16, tag="w", name="wq") for _ in range(HT)]
            wk_sb = [wp.tile([P, H], f16, tag="w", name="wk") for _ in range(HT)]
            wv_sb = [wp.tile([P, H], f16, tag="w", name="wv") for _ in range(HT)]
            for j in range(HT):
                sl = slice(j * P, (j + 1) * P)
                nc.gpsimd.dma_start(xdt_sb[j][:], xdt_d[sl, :])
                nc.gpsimd.dma_start(xet_sb[j][:], xet_d[sl, :])
                nc.gpsimd.dma_start(wq_sb[j][:], wqt_d[sl, :])
                nc.gpsimd.dma_start(wk_sb[j][:], wkt_d[sl, :])
                nc.gpsimd.dma_start(wv_sb[j][:], wvt_d[sl, :])

            # ---- projection prep chunks (woven into the pair loop) ----
            q_t = {}
            k_t = {}
            v2_next = [None]

            def _proj_half(w_sb, x_sb, dst, p, nn, on_act):
                # one s-half of a [128, S] projection in a [128,512] bank
                psq = pstile()
                for j in range(HT):
                    nc.tensor.matmul(
                        psq[:], w_sb[j][:, p * P:(p + 1) * P],
                        x_sb[j][:, nn * 512:(nn + 1) * 512],
                        start=(j == 0), stop=(j == HT - 1))
                sl = slice(nn * 512, (nn + 1) * 512)
                if on_act:
                    nc.scalar.copy(dst[:, sl], psq[:])
                else:
                    nc.vector.tensor_copy(dst[:, sl], psq[:])

            def q_chunks(p):
                qt = qtp.tile([P, S], f16, tag="qt", name="qt")
                q_t[p] = qt

                def mk(nn):
                    def c():
                        _proj_half(wq_sb, xdt_sb, qt, p, nn, nn == 0)
                    return c
                return [mk(0), mk(1)]

            def k_chunks(p):
                kt = ktp.tile([P, S], f16, tag="kt", name="kt")
                k_t[p] = kt

                def mk(nn):
                    def c():
                        _proj_half(wk_sb, xet_sb, kt, p, nn, nn == 1)
                    return c
                return [mk(0), mk(1)]

            def v_chunks(p):
                # p is the first pair of a 4-pair group; covers nd cols
                # [p*128, (p+4)*128) = 512 of wvt
                v2_box = [None]

                def mk(kt_i):
                    def cg():
                        if v2_box[0] is None:
                            v2_box[0] = vpp.tile([P, ST, 512], f16, tag="vp",
                                                 name="v2")
                            v2_next[0] = v2_box[0]
                        v2n = v2_box[0]
                        psv = pstile()
                        for j in range(HT):
                            nc.tensor.matmul(
                                psv[:],
                                xet_sb[j][:, kt_i * P:(kt_i + 1) * P],
                                wv_sb[j][:, p * P:(p + 4) * P],
                                start=(j == 0), stop=(j == HT - 1))
                        nc.vector.tensor_copy(v2n[:, kt_i, :], psv[:])
                    return cg

                return [mk(kt_i) for kt_i in range(ST)]

            # prologue: Q for pairs 0-1, K for pair 0, V for pairs 0-3
            for c in q_chunks(0) + q_chunks(1):
                c()
            for c in k_chunks(0):
                c()
            for c in v_chunks(0):
                c()
            v2 = v2_next[0]

            concat_t = []
            vch_cache = {}
            for p in range(NP):
                chunks = []
                if p + 2 < NP:
                    chunks += q_chunks(p + 2)
                if p + 1 < NP:
                    chunks += k_chunks(p + 1)
                # two V chunks per pair, spread over the 4 preceding pairs
                G = (p // 4 + 1) * 4
                if G < NP:
                    if G not in vch_cache:
                        vch_cache[G] = v_chunks(G)
                    chunks.append(vch_cache[G][(p % 4) * 2])
                    chunks.append(vch_cache[G][(p % 4) * 2 + 1])
                vc = (p % 4) * P

                qt = q_t[p]
                kt = k_t[p]
                concat = ccp.tile([P, S], f16, tag="cc", name="concat")
                concat_t.append(concat)

                pending_pv = [None]

                def emit_pv(args):
                    v2_, vc_, pt_ev_, pt_od_, concat_, blk_ = args
                    ps_o = pstile()
                    for kt_i in range(ST):
                        nc.tensor.matmul(
                            ps_o[0:64, 0:QB],
                            v2_[:, kt_i, vc_:vc_ + 64],
                            pt_ev_[:, kt_i, :],
                            start=(kt_i == 0), stop=(kt_i == ST - 1),
                            tile_position=(0, 0))
                        nc.tensor.matmul(
                            ps_o[64:128, 0:QB],
                            v2_[:, kt_i, vc_ + 64:vc_ + 128],
                            pt_od_[:, kt_i, :],
                            start=(kt_i == 0), stop=(kt_i == ST - 1),
                            tile_position=(0, 64))
                    nc.vector.tensor_copy(
                        concat_[:, blk_ * QB:(blk_ + 1) * QB],
                        ps_o[:, 0:QB])

                for blk in range(NB):
                    pt_ev = ptp.tile([P, ST, QB], f16, tag="pt", name="ptev")
                    pt_od = ptp.tile([P, ST, QB], f16, tag="pt", name="ptod")
                    for qtb in range(QTB):
                        qti = blk * QTB + qtb
                        qs = slice(qti * P, (qti + 1) * P)
                        ps_s = [pstileS(), pstileS()]
                        # interleave ev/od so adjacent matmuls occupy
                        # disjoint PE row groups
                        for kk in range(2):
                            ks = slice(kk * 512, (kk + 1) * 512)
                            for h01 in range(2):
                                base = h01 * 64
                                nc.tensor.matmul(
                                    ps_s[h01][:, ks],
                                    qt[base:base + 64, qs],
                                    kt[base:base + 64, ks],
                                    start=True, stop=True,
                                    tile_position=(base, 0))
                        if pending_pv[0] is not None:
                            emit_pv(pending_pv[0])
                            pending_pv[0] = None
                        elif chunks:
                            chunks.pop(0)()
                            if chunks:
                                chunks.pop(0)()
                        for h01 in range(2):
                            pt_dst = pt_ev if h01 == 0 else pt_od
                            negmax, rsum, recip = stat(), stat(), stat()
                            nc.vector.tensor_reduce(
                                negmax[:], ps_s[h01][:], axis=AX,
                                op=OP.max, negate=True)
                            p_e = pep.tile([P, S], f16, tag="pe")
                            nc.scalar.activation(
                                p_e[:], ps_s[h01][:], AF.Exp,
                                bias=negmax[:], accum_out=rsum[:])
                            nc.vector.reciprocal(recip[:], rsum[:])
                            p_h = php.tile([P, S], f16, tag="ph")
                            if h01 == 0:
                                nc.vector.tensor_scalar_mul(
                                    p_h[:], p_e[:], recip[:])
                            else:
                                nc.gpsimd.tensor_scalar_mul(
                                    p_h[:], p_e[:], recip[:])
                            nc.sync.dma_start_transpose(
                                pt_dst[:, :, qtb * P:(qtb + 1) * P], p_h[:])
                    pending_pv[0] = (v2, vc, pt_ev, pt_od, concat, blk)
                # drain: last block PV + any remaining prep chunks
                emit_pv(pending_pv[0])
                for c in chunks:
                    c()
                if (p + 1) % 4 == 0 and p + 1 < NP:
                    v2 = v2_next[0]

            # ---- phase D: out = concat @ W_out^T + b ----
            for sg in range(2):
                ps_big = [pstileS(), pstileS(), pstileS()]
                ps_sm = [pstile(), pstile()]

                def out_slot(sl, half):
                    # slots: 3 [128,1024] tiles (6 halves) + 2 [128,512]
                    idx = sl * 2 + half
                    if idx < 6:
                        return ps_big[idx // 2][:, (idx % 2) * 512:
                                                (idx % 2) * 512 + 512]
                    return ps_sm[idx - 6][:]

                for p in range(NP):
                    wo_r = []
                    for half in range(2):
                        wo_sb = wop.tile([P, 512], f16, tag="wo")
                        nc.gpsimd.dma_start(
                            wo_sb[:],
                            wot_d[p * P:(p + 1) * P,
                                  half * 512:(half + 1) * 512])
                        wo_r.append(wo_sb)
                    for sl in range(4):
                        st = sg * 4 + sl
                        for half in range(2):
                            nc.tensor.matmul(
                                out_slot(sl, half),
                                concat_t[p][:, st * P:(st + 1) * P],
                                wo_r[half][:],
                                start=(p == 0), stop=(p == NP - 1))
                for sl in range(4):
                    st = sg * 4 + sl
                    out_sb = osbp.tile([P, H], f32, tag="osb")
                    for half in range(2):
                        nc.vector.tensor_tensor(
                            out_sb[:, half * 512:(half + 1) * 512],
                            out_slot(sl, half),
                            bias_sb[:, half * 512:(half + 1) * 512],
                            op=OP.add)
                    nc.scalar.dma_start(out_d[st * P:(st + 1) * P, :], out_sb[:])

    nc.compile()
    return nc


def prep_in_maps(decoder_input, encoder_output, W_query, W_key, W_value,
                 W_out, b_out):
    f = lambda a: np.asarray(a, dtype=np.float32)
    di = f(decoder_input)
    eo = f(encoder_output)
    wq = (f(W_query).reshape(H, H) * np.float32(0.125)).T.astype(np.float16)
    wk = f(W_key).reshape(H, H).T.astype(np.float16)
    wv = f(W_value).reshape(H, H).T.astype(np.float16)
    wo = f(W_out).T.astype(np.float16)
    bias = np.ascontiguousarray(np.broadcast_to(f(b_out), (P, H)))
    return [
        {"xdt": di[b].T.astype(np.float16), "xet": eo[b].T.astype(np.float16),
         "wqt": wq, "wkt": wk, "wvt": wv, "wot": wo, "bias": bias}
        for b in range(B)
    ]


_BUILT = None


def kernel(decoder_input, encoder_output, W_query, W_key, W_value, W_out,
           b_out):
    global _BUILT
    from concourse import bass_utils
    if _BUILT is None:
        _BUILT = build()
    in_maps = prep_in_maps(decoder_input, encoder_output, W_query, W_key,
                           W_value, W_out, b_out)
    try:
        res = bass_utils.run_bass_kernel_spmd(_BUILT, in_maps,
                                              core_ids=list(range(B)))
    except Exception:
        # one retry: a previously wedged NeuronCore can fail the first
        # execution after load
        res = bass_utils.run_bass_kernel_spmd(_BUILT, in_maps,
                                              core_ids=list(range(B)))
    return np.stack([res.results[b]["out"] for b in range(B)], axis=0)


# revision 15
# speedup vs baseline: 2.2920x; 2.2920x over previous
"""Encoder-decoder attention kernel for Trainium2, 8 NeuronCores.

Sharding: batch (B=8) data-parallel, one batch element per core; weights
replicated. Per core (S=Sq=Sk=1024, H=1024, NH=16, D=64):

All matmuls run in fp16 (1 cycle/row on PE) with fp32 PSUM accumulation.
Host ships X_dec^T, X_enc^T and all weights pre-transposed + pre-cast to
fp16 (1/sqrt(D)=1/8 folded into W_query), so there is no on-chip
transpose/cast phase for the inputs.

Per head-pair p (heads 2p, 2p+1 stacked as 64+64 partitions):
  Q^T = Wq_p^T.T @ X_dec^T  -> PSUM -> fp16 qt [128, S]
  K^T -> fp16 kt [128, S];  V -> v2 [128 k, kt, 512 nd] fp16 (per 4-pair grp)
  scores: contraction-64 fp16 matmuls, ev head on PE rows 0-63
    (tile_position (0,0)) and od head on rows 64-127 ((64,0)) so the two
    heads' streams overlap in the array.
  softmax per [128 q, 1024 k] tile:
    DVE tensor_tensor_reduce: out=-max(s_lo,s_hi) elemwise (fp16 scratch),
      min-accum -> negmax (one 512-elem pass instead of a 1024-elem reduce)
    ACT exp(bias=negmax, accum_out=rowsum) -> p_e fp16
    DVE reciprocal + tensor_scalar_mul (4x fp16) -> p_h
    sync DMA transpose p_h -> pt [128 k, kt, q]
  PV: fp16, ev/od col-split via tile_position (0,0)/(0,64)
out = concat @ W_out^T + b_out (fp16 matmuls, DVE bias add).

Error budget: fp16 rounding of q/k gives score error ~0.1 abs (scores are
~N(0, 2000^2), top-2 gaps ~400); measured end-to-end rel l2 ~1.5e-3 vs
the 2e-2 gate.
"""
import sys

sys.path.insert(0, "/opt/trn_rl_repo")

import numpy as np

B = 8
S = 1024   # Sq == Sk
H = 1024
NH = 16
D = 64
P = 128
HT = H // P    # 8 h-tiles
ST = S // P    # 8 s-tiles == k-tiles
NP = NH // 2   # 8 head pairs
QB = 256       # q-block width for the P@V moving dim
NB = S // QB   # 4 q-blocks
QTB = QB // P  # 2 q-tiles per block


def build():
    import concourse.mybir as mybir
    import concourse.tile as tile
    from concourse import bacc

    f32 = mybir.dt.float32
    f16 = mybir.dt.float16
    AX = mybir.AxisListType.X
    OP = mybir.AluOpType
    AF = mybir.ActivationFunctionType

    nc = bacc.Bacc(trn_type="TRN2", target_bir_lowering=False, debug=False)

    xdt_d = nc.dram_tensor("xdt", [H, S], f16, kind="ExternalInput").ap()   # X_dec^T
    xet_d = nc.dram_tensor("xet", [H, S], f16, kind="ExternalInput").ap()   # X_enc^T
    wqt_d = nc.dram_tensor("wqt", [H, H], f16, kind="ExternalInput").ap()   # [h, nd] (pre-scaled 1/8)
    wkt_d = nc.dram_tensor("wkt", [H, H], f16, kind="ExternalInput").ap()   # [h, nd]
    wvt_d = nc.dram_tensor("wvt", [H, H], f16, kind="ExternalInput").ap()   # [h, nd]
    wot_d = nc.dram_tensor("wot", [H, H], f16, kind="ExternalInput").ap()   # [nd, h_out]
    bias_d = nc.dram_tensor("bias", [P, H], f32, kind="ExternalInput").ap()
    out_d = nc.dram_tensor("out", [S, H], f32, kind="ExternalOutput").ap()

    from contextlib import ExitStack
    with tile.TileContext(nc) as tc:
        with ExitStack() as ctx:
            xp = ctx.enter_context(tc.tile_pool(name="x", bufs=16))
            wp = ctx.enter_context(tc.tile_pool(name="w", bufs=24))
            qtp = ctx.enter_context(tc.tile_pool(name="qt", bufs=3))
            ktp = ctx.enter_context(tc.tile_pool(name="kt", bufs=3))
            vpp = ctx.enter_context(tc.tile_pool(name="vp", bufs=2))
            ccp = ctx.enter_context(tc.tile_pool(name="cc", bufs=NP))
            pep = ctx.enter_context(tc.tile_pool(name="pe", bufs=6))
            php = ctx.enter_context(tc.tile_pool(name="ph", bufs=6))
            ptp = ctx.enter_context(tc.tile_pool(name="pt", bufs=6))
            wop = ctx.enter_context(tc.tile_pool(name="wo", bufs=4))
            osbp = ctx.enter_context(tc.tile_pool(name="osb", bufs=2))
            constp = ctx.enter_context(tc.tile_pool(name="const", bufs=1))
            statp = ctx.enter_context(tc.tile_pool(name="stat", bufs=24))
            psp = ctx.enter_context(tc.tile_pool(name="ps", bufs=2, space="PSUM"))
            psSp = ctx.enter_context(tc.tile_pool(name="psS", bufs=3, space="PSUM"))

            def pstile():
                return psp.tile([P, 512], f32, tag="ps", name="ps")

            def pstileS():
                return psSp.tile([P, S], f32, tag="psS", name="psS")

            def stat():
                return statp.tile([P, 1], f32, tag="stat", name="stat")

            # ---- constants / preloaded inputs ----
            bias_sb = constp.tile([P, H], f32)
            nc.gpsimd.dma_start(bias_sb[:], bias_d)

            xdt_sb = [xp.tile([P, S], f16, tag="x", name="xdt") for _ in range(HT)]
            xet_sb = [xp.tile([P, S], f16, tag="x", name="xet") for _ in range(HT)]
            wq_sb = [wp.tile([P, H], f16, tag="w", name="wq") for _ in range(HT)]
            wk_sb = [wp.tile([P, H], f16, tag="w", name="wk") for _ in range(HT)]
            wv_sb = [wp.tile([P, H], f16, tag="w", name="wv") for _ in range(HT)]
            for j in range(HT):
                sl = slice(j * P, (j + 1) * P)
                nc.gpsimd.dma_start(xdt_sb[j][:], xdt_d[sl, :])
                nc.gpsimd.dma_start(xet_sb[j][:], xet_d[sl, :])
                nc.gpsimd.dma_start(wq_sb[j][:], wqt_d[sl, :])
                nc.gpsimd.dma_start(wk_sb[j][:], wkt_d[sl, :])
                nc.gpsimd.dma_start(wv_sb[j][:], wvt_d[sl, :])

            # ---- projection prep chunks (woven into the pair loop) ----
            q_t = {}
            k_t = {}
            v2_next = [None]

            def _proj_half(w_sb, x_sb, dst, p, nn, on_act):
                # one s-half of a [128, S] projection in a [128,512] bank
                psq = pstile()
                for j in range(HT):
                    nc.tensor.matmul(
                        psq[:], w_sb[j][:, p * P:(p + 1) * P],
                        x_sb[j][:, nn * 512:(nn + 1) * 512],
                        start=(j == 0), stop=(j == HT - 1))
                sl = slice(nn * 512, (nn + 1) * 512)
                if on_act:
                    nc.scalar.copy(dst[:, sl], psq[:])
                else:
                    nc.vector.tensor_copy(dst[:, sl], psq[:])

            def q_chunks(p):
                qt = qtp.tile([P, S], f16, tag="qt", name="qt")
                q_t[p] = qt

                def mk(nn):
                    def c():
                        _proj_half(wq_sb, xdt_sb, qt, p, nn, nn == 0)
                    return c
                return [mk(0), mk(1)]

            def k_chunks(p):
                kt = ktp.tile([P, S], f16, tag="kt", name="kt")
                k_t[p] = kt

                def mk(nn):
                    def c():
                        _proj_half(wk_sb, xet_sb, kt, p, nn, nn == 1)
                    return c
                return [mk(0), mk(1)]

            def v_chunks(p):
                # p is the first pair of a 4-pair group; covers nd cols
                # [p*128, (p+4)*128) = 512 of wvt
                v2_box = [None]

                def mk(kt_i):
                    def cg():
                        if v2_box[0] is None:
                            v2_box[0] = vpp.tile([P, ST, 512], f16, tag="vp",
                                                 name="v2")
                            v2_next[0] = v2_box[0]
                        v2n = v2_box[0]
                        psv = pstile()
                        for j in range(HT):
                            nc.tensor.matmul(
                                psv[:],
                                xet_sb[j][:, kt_i * P:(kt_i + 1) * P],
                                wv_sb[j][:, p * P:(p + 4) * P],
                                start=(j == 0), stop=(j == HT - 1))
                        nc.vector.tensor_copy(v2n[:, kt_i, :], psv[:])
                    return cg

                return [mk(kt_i) for kt_i in range(ST)]

            # prologue: Q/K for pair 0 + half of V group 0; the rest of
            # V group 0 fills pair 0's first weave slots (they must all
            # precede pair 0's first PV emit in program order), then Q1
            for c in q_chunks(0) + k_chunks(0):
                c()
            v0_chunks = v_chunks(0)
            for c in v0_chunks[:4]:
                c()
            v2 = v2_next[0]

            concat_t = []
            vch_cache = {}
            for p in range(NP):
                chunks = []
                if p == 0:
                    chunks += v0_chunks[4:]
                    chunks += q_chunks(1)
                if p + 2 < NP:
                    chunks += q_chunks(p + 2)
                if p + 1 < NP:
                    chunks += k_chunks(p + 1)
                # two V chunks per pair, spread over the 4 preceding pairs
                G = (p // 4 + 1) * 4
                if G < NP:
                    if G not in vch_cache:
                        vch_cache[G] = v_chunks(G)
                    chunks.append(vch_cache[G][(p % 4) * 2])
                    chunks.append(vch_cache[G][(p % 4) * 2 + 1])
                vc = (p % 4) * P

                qt = q_t[p]
                kt = k_t[p]
                concat = ccp.tile([P, S], f16, tag="cc", name="concat")
                concat_t.append(concat)

                pending_pv = [None]

                def emit_pv(args):
                    v2_, vc_, pt_ev_, pt_od_, concat_, blk_ = args
                    ps_o = pstile()
                    for kt_i in range(ST):
                        nc.tensor.matmul(
                            ps_o[0:64, 0:QB],
                            v2_[:, kt_i, vc_:vc_ + 64],
                            pt_ev_[:, kt_i, :],
                            start=(kt_i == 0), stop=(kt_i == ST - 1),
                            tile_position=(0, 0))
                        nc.tensor.matmul(
                            ps_o[64:128, 0:QB],
                            v2_[:, kt_i, vc_ + 64:vc_ + 128],
                            pt_od_[:, kt_i, :],
                            start=(kt_i == 0), stop=(kt_i == ST - 1),
                            tile_position=(0, 64))
                    nc.vector.tensor_copy(
                        concat_[:, blk_ * QB:(blk_ + 1) * QB],
                        ps_o[:, 0:QB])

                for blk in range(NB):
                    pt_ev = ptp.tile([P, ST, QB], f16, tag="pt", name="ptev")
                    pt_od = ptp.tile([P, ST, QB], f16, tag="pt", name="ptod")
                    for qtb in range(QTB):
                        qti = blk * QTB + qtb
                        qs = slice(qti * P, (qti + 1) * P)
                        ps_s = [pstileS(), pstileS()]
                        # interleave ev/od so adjacent matmuls occupy
                        # disjoint PE row groups
                        for kk in range(2):
                            ks = slice(kk * 512, (kk + 1) * 512)
                            for h01 in range(2):
                                base = h01 * 64
                                nc.tensor.matmul(
                                    ps_s[h01][:, ks],
                                    qt[base:base + 64, qs],
                                    kt[base:base + 64, ks],
                                    start=True, stop=True,
                                    tile_position=(base, 0))
                        if pending_pv[0] is not None:
                            emit_pv(pending_pv[0])
                            pending_pv[0] = None
                        elif chunks:
                            chunks.pop(0)()
                            if chunks:
                                chunks.pop(0)()
                        for h01 in range(2):
                            pt_dst = pt_ev if h01 == 0 else pt_od
                            negmax, rsum, recip = stat(), stat(), stat()
                            nc.vector.tensor_reduce(
                                negmax[:], ps_s[h01][:], axis=AX,
                                op=OP.max, negate=True)
                            p_e = pep.tile([P, S], f16, tag="pe")
                            nc.scalar.activation(
                                p_e[:], ps_s[h01][:], AF.Exp,
                                bias=negmax[:], accum_out=rsum[:])
                            nc.vector.reciprocal(recip[:], rsum[:])
                            p_h = php.tile([P, S], f16, tag="ph")
                            nc.vector.tensor_scalar_mul(
                                p_h[:], p_e[:], recip[:])
                            nc.sync.dma_start_transpose(
                                pt_dst[:, :, qtb * P:(qtb + 1) * P], p_h[:])
                    pending_pv[0] = (v2, vc, pt_ev, pt_od, concat, blk)
                # drain: last block PV + any remaining prep chunks
                emit_pv(pending_pv[0])
                for c in chunks:
                    c()
                if (p + 1) % 4 == 0 and p + 1 < NP:
                    v2 = v2_next[0]

            # ---- phase D: out = concat @ W_out^T + b ----
            for sg in range(2):
                ps_big = [pstileS(), pstileS(), pstileS()]
                ps_sm = [pstile(), pstile()]

                def out_slot(sl, half):
                    # slots: 3 [128,1024] tiles (6 halves) + 2 [128,512]
                    idx = sl * 2 + half
                    if idx < 6:
                        return ps_big[idx // 2][:, (idx % 2) * 512:
                                                (idx % 2) * 512 + 512]
                    return ps_sm[idx - 6][:]

                for p in range(NP):
                    wo_r = []
                    for half in range(2):
                        wo_sb = wop.tile([P, 512], f16, tag="wo")
                        nc.gpsimd.dma_start(
                            wo_sb[:],
                            wot_d[p * P:(p + 1) * P,
                                  half * 512:(half + 1) * 512])
                        wo_r.append(wo_sb)
                    for sl in range(4):
                        st = sg * 4 + sl
                        for half in range(2):
                            nc.tensor.matmul(
                                out_slot(sl, half),
                                concat_t[p][:, st * P:(st + 1) * P],
                                wo_r[half][:],
                                start=(p == 0), stop=(p == NP - 1))
                for sl in range(4):
                    st = sg * 4 + sl
                    out_sb = osbp.tile([P, H], f32, tag="osb")
                    for half in range(2):
                        nc.vector.tensor_tensor(
                            out_sb[:, half * 512:(half + 1) * 512],
                            out_slot(sl, half),
                            bias_sb[:, half * 512:(half + 1) * 512],
                            op=OP.add)
                    nc.scalar.dma_start(out_d[st * P:(st + 1) * P, :], out_sb[:])

    nc.compile()
    return nc


def prep_in_maps(decoder_input, encoder_output, W_query, W_key, W_value,
                 W_out, b_out):
    f = lambda a: np.asarray(a, dtype=np.float32)
    di = f(decoder_input)
    eo = f(encoder_output)
    wq = (f(W_query).reshape(H, H) * np.float32(0.125)).T.astype(np.float16)
    wk = f(W_key).reshape(H, H).T.astype(np.float16)
    wv = f(W_value).reshape(H, H).T.astype(np.float16)
    wo = f(W_out).T.astype(np.float16)
    bias = np.ascontiguousarray(np.broadcast_to(f(b_out), (P, H)))
    return [
        {"xdt": di[b].T.astype(np.float16), "xet": eo[b].T.astype(np.float16),
         "wqt": wq, "wkt": wk, "wvt": wv, "wot": wo, "bias": bias}
        for b in range(B)
    ]


_BUILT = None


def kernel(decoder_input, encoder_output, W_query, W_key, W_value, W_out,
           b_out):
    global _BUILT
    from concourse import bass_utils
    if _BUILT is None:
        _BUILT = build()
    in_maps = prep_in_maps(decoder_input, encoder_output, W_query, W_key,
                           W_value, W_out, b_out)
    try:
        res = bass_utils.run_bass_kernel_spmd(_BUILT, in_maps,
                                              core_ids=list(range(B)))
    except Exception:
        # one retry: a previously wedged NeuronCore can fail the first
        # execution after load
        res = bass_utils.run_bass_kernel_spmd(_BUILT, in_maps,
                                              core_ids=list(range(B)))
    return np.stack([res.results[b]["out"] for b in range(B)], axis=0)
